# revision 1
# baseline (speedup 1.0000x reference)
"""Trainium2 Bass kernel v4 for nn_Attention (B=1, N=4096, DIM=768, HEADS=12).

v1's attention inner loop (128-row zero-padded score tiles, bf16 V/e,
ones-column softmax denominators, exp on the Act engine at ~1.05us
spacing) is kept; the projection phase is sharded 8x across cores and
exchanged via AllGather (~100-200 GB/s measured).

v4 vs v3: K/Q in fp16 (10-bit mantissa, numerically equivalent to
f32r here; halves K gather bytes + kt SBUF + kt load time), V gathered
in a [head, partition, seqtile, d] per-pair layout so phase-B V loads
move 520B lines instead of 130B, collectives interleaved with staging
on the gpsimd queue (K pair 0 gathered first, then V, then K 1-5),
pair-0 Q projected before V so scores can start at ~45us, explicit
one-pair-ahead kt/V prefetch and a 6-deep exp ring to ride out
collective jitter.
"""

import os
import sys
from contextlib import ExitStack

import numpy as np

sys.path.insert(0, "/opt/trn_rl_repo")

import concourse.bass as bass  # noqa: E402
import concourse.tile as tile  # noqa: E402
from concourse import bacc, mybir  # noqa: E402
from concourse.bass_utils import run_bass_kernel_spmd  # noqa: E402

N_CORES = 8
DIM = 768
HEADS = 12
SEQ = 4096
DHEAD = 64
NQ = SEQ // N_CORES  # 512 queries per core
NSH = SEQ // N_CORES  # 512 keys/values projected per core (own chunk)
NPAIRS = HEADS // 2  # 6 head pairs
KT = DIM // 128  # 6 contraction tiles
F32 = mybir.dt.float32
F32R = mybir.dt.float32r
F16 = mybir.dt.float16
BF16 = mybir.dt.bfloat16

_CACHE = {}


def _build():
    nc = bacc.Bacc("TRN2", target_bir_lowering=False, debug=False, num_devices=N_CORES)

    xcT = nc.dram_tensor("xcT", [KT, 128, NSH], F32R, kind="ExternalInput").ap()
    xT16 = nc.dram_tensor("xT16", [KT, 128, SEQ], F16, kind="ExternalInput").ap()
    wk01 = nc.dram_tensor("wk01", [KT, 128, 256], F16, kind="ExternalInput").ap()
    wv01 = nc.dram_tensor("wv01", [KT, 128, 256], F16, kind="ExternalInput").ap()
    wq = nc.dram_tensor("wq", [KT, 128, DIM], F32R, kind="ExternalInput").ap()
    wk = nc.dram_tensor("wk", [KT, 128, DIM], F32R, kind="ExternalInput").ap()
    wv = nc.dram_tensor("wv", [KT, 128, DIM], F32R, kind="ExternalInput").ap()
    wo = nc.dram_tensor("wo", [NPAIRS, 128, DIM], BF16, kind="ExternalInput").ap()
    bo = nc.dram_tensor("bo", [DIM], F32, kind="ExternalInput").ap()
    out = nc.dram_tensor("out", [NQ, DIM], F32, kind="ExternalOutput").ap()

    # collective bounce buffers for pairs 2-5 (pairs 0-1 are computed
    # redundantly full-seq while the CC engine warms up ~100us)
    agk_in = {j: nc.dram_tensor(f"agk_in{j}", [128, NSH], F16).ap() for j in range(2, NPAIRS)}
    agk_out = {
        j: nc.dram_tensor(f"agk_out{j}", [N_CORES, 128, NSH], F16, addr_space="Shared").ap()
        for j in range(2, NPAIRS)
    }
    agv_in = {
        j: nc.dram_tensor(f"agv_in{j}", [128, 2, 4, DHEAD + 1], BF16).ap()
        for j in range(2, NPAIRS)
    }
    agv_out = {
        j: nc.dram_tensor(
            f"agv_out{j}", [N_CORES, 128, 2, 4, DHEAD + 1], BF16, addr_space="Shared"
        ).ap()
        for j in range(2, NPAIRS)
    }
    groups = [list(range(N_CORES))]

    with ExitStack() as ctx:
        tc = ctx.enter_context(tile.TileContext(nc))

        persist = ctx.enter_context(tc.tile_pool(name="persist", bufs=1))
        vpool = ctx.enter_context(tc.tile_pool(name="vpool", bufs=3))
        qT_sb = [persist.tile([128, NQ], F16, tag=f"qt{h}", name=f"qt{h}") for h in range(HEADS)]
        proj_sb = [persist.tile([128, NQ], BF16, tag=f"proj{j}", name=f"proj{j}") for j in range(NPAIRS)]
        # persistent K tiles (ping/pong per head-of-pair), upper halves zeroed
        ktiles = [persist.tile([128, SEQ], F16, tag=f"ktile{i}", name=f"ktile{i}") for i in range(2)]
        ktiles2 = [persist.tile([128, SEQ], F16, tag=f"ktile2_{i}", name=f"ktile2_{i}") for i in range(2)]
        with ExitStack() as zctx:
            zpool = zctx.enter_context(tc.tile_pool(name="zpool", bufs=1))
            zero_f = zpool.tile([64, SEQ], F32, tag="zero_f", name="zero_f")
            nc.vector.memset(zero_f[:], 0.0)
            zero_h = zpool.tile([64, SEQ], F16, tag="zero_h", name="zero_h")
            nc.vector.tensor_copy(zero_h[:], zero_f[:])
            for h in range(HEADS):
                nc.vector.tensor_copy(qT_sb[h][DHEAD:128, :], zero_h[:, 0:NQ])
            for i in range(2):
                nc.vector.tensor_copy(ktiles[i][DHEAD:128, :], zero_h[:])
                nc.vector.tensor_copy(ktiles2[i][DHEAD:128, :], zero_h[:])

        # ---------------- Phase A: sharded projections + allgather ----------------
        with ExitStack() as pa:
            wpool = pa.enter_context(tc.tile_pool(name="wpool", bufs=1))
            evac = pa.enter_context(tc.tile_pool(name="evac", bufs=3))
            wqp = pa.enter_context(tc.tile_pool(name="wqp", bufs=3))
            psK = pa.enter_context(tc.tile_pool(name="psK", bufs=2, space="PSUM"))
            psV = pa.enter_context(tc.tile_pool(name="psV", bufs=2, space="PSUM"))
            psQ = pa.enter_context(tc.tile_pool(name="psQ", bufs=2, space="PSUM"))

            x16_sb = [wpool.tile([128, SEQ], F16, tag=f"x16_{k}", name=f"x16_{k}") for k in range(KT)]
            wk01_sb = [wpool.tile([128, 256], F16, tag=f"wk01_{k}", name=f"wk01_{k}") for k in range(KT)]
            wv01_sb = [wpool.tile([128, 256], F16, tag=f"wv01_{k}", name=f"wv01_{k}") for k in range(KT)]
            for k in range(KT):
                nc.sync.dma_start(out=wk01_sb[k][:], in_=wk01[k])
                nc.sync.dma_start(out=wv01_sb[k][:], in_=wv01[k])
                nc.sync.dma_start(out=x16_sb[k][:], in_=xT16[k])
            xc_sb = [wpool.tile([128, NSH], F32R, tag=f"xc{k}", name=f"xc{k}") for k in range(KT)]
            wk_sb = [wpool.tile([128, DIM], F32R, tag=f"wk{k}", name=f"wk{k}") for k in range(KT)]
            wv_sb = [wpool.tile([128, DIM], F32R, tag=f"wv{k}", name=f"wv{k}") for k in range(KT)]
            for k in range(KT):
                nc.sync.dma_start(out=xc_sb[k][:], in_=xcT[k])
                nc.sync.dma_start(out=wk_sb[k][:], in_=wk[k])
            for k in range(KT):
                nc.sync.dma_start(out=wv_sb[k][:], in_=wv[k])

            def _kproj(j):
                ps = psK.tile([128, NSH], F32, tag="psk", name="psk")
                for k in range(KT):
                    nc.tensor.matmul(
                        ps[:], wk_sb[k][:, j * 128:(j + 1) * 128], xc_sb[k][:],
                        start=(k == 0), stop=(k == KT - 1),
                    )
                kev = evac.tile([128, NSH], F16, tag="kev", name="kev")
                nc.vector.tensor_copy(kev[:], ps[:])
                nc.gpsimd.dma_start(out=agk_in[j], in_=kev[:])
                nc.gpsimd.collective_compute(
                    "AllGather", mybir.AluOpType.bypass, replica_groups=groups,
                    ins=[agk_in[j].opt()], outs=[agk_out[j].opt()],
                )

            def _qproj(mt):
                wq_mt = wqp.tile([128, KT, 128], F32R, tag="wq_mt", name="wq_mt")
                for k in range(KT):
                    nc.sync.dma_start(
                        out=wq_mt[:, k, :],
                        in_=wq[k][:, mt * 128:(mt + 1) * 128],
                    )
                ps = psQ.tile([128, NQ], F32, tag="psq", name="psq")
                for k in range(KT):
                    nc.tensor.matmul(
                        ps[:], wq_mt[:, k, :], xc_sb[k][:],
                        start=(k == 0), stop=(k == KT - 1),
                    )
                qev = evac.tile([128, NQ], F16, tag="qev", name="qev")
                nc.vector.tensor_copy(qev[:], ps[:])
                nc.gpsimd.dma_start(out=qT_sb[2 * mt][0:DHEAD, :], in_=qev[0:DHEAD, :])
                nc.gpsimd.dma_start(out=qT_sb[2 * mt + 1][0:DHEAD, :], in_=qev[DHEAD:128, :])

            # ---- redundant full-seq pairs 0-1 (fp16) during CC warm-up ----
            def _kfull(j):
                kt1 = ktiles[j % 2]
                kt2 = ktiles2[j % 2]
                for nch in range(8):
                    ps = psK.tile([128, 512], F32, tag="psk", name="psk")
                    for k in range(KT):
                        nc.tensor.matmul(
                            ps[:], wk01_sb[k][:, j * 128:(j + 1) * 128],
                            x16_sb[k][:, nch * 512:(nch + 1) * 512],
                            start=(k == 0), stop=(k == KT - 1),
                        )
                    kev = evac.tile([128, 512], F16, tag="kev", name="kev")
                    nc.vector.tensor_copy(kev[:], ps[:])
                    nc.gpsimd.dma_start(
                        out=kt1[0:DHEAD, nch * 512:(nch + 1) * 512], in_=kev[0:DHEAD, :])
                    nc.gpsimd.dma_start(
                        out=kt2[0:DHEAD, nch * 512:(nch + 1) * 512], in_=kev[DHEAD:128, :])

            _kfull(0)
            _qproj(0)

            # V pairs 0-1 full sequence straight into SBUF v tiles
            v1_0 = vpool.tile([128, SEQ // 128, DHEAD + 1], BF16, tag="v1", name="v1_p0")
            v2_0 = vpool.tile([128, SEQ // 128, DHEAD + 1], BF16, tag="v2", name="v2_p0")
            v1_1 = vpool.tile([128, SEQ // 128, DHEAD + 1], BF16, tag="v1", name="v1_p1")
            v2_1 = vpool.tile([128, SEQ // 128, DHEAD + 1], BF16, tag="v2", name="v2_p1")
            for t in (v1_0, v2_0, v1_1, v2_1):
                nc.vector.memset(t[:, :, DHEAD:DHEAD + 1], 1.0)
            for st in range(SEQ // 128):
                ps = psV.tile([128, 256], F32, tag="psv01", name="psv01")
                for k in range(KT):
                    nc.tensor.matmul(
                        ps[:], x16_sb[k][:, st * 128:(st + 1) * 128], wv01_sb[k][:],
                        start=(k == 0), stop=(k == KT - 1),
                    )
                nc.vector.tensor_copy(v1_0[:, st, 0:DHEAD], ps[:, 0:64])
                nc.vector.tensor_copy(v2_0[:, st, 0:DHEAD], ps[:, 64:128])
                nc.vector.tensor_copy(v1_1[:, st, 0:DHEAD], ps[:, 128:192])
                nc.vector.tensor_copy(v2_1[:, st, 0:DHEAD], ps[:, 192:256])

            _kfull(1)
            _qproj(1)

            # ---- sharded projections + gathers for pairs 2-5 ----
            # V for own chunk (head cols 256:768) -> per-pair staging
            for st in range(4):
                ps = psV.tile([128, 512], F32, tag="psv", name="psv")
                for k in range(KT):
                    lhs = xc_sb[k][:, st * 128:(st + 1) * 128]
                    nc.tensor.matmul(ps[:], lhs, wv_sb[k][:, 256:DIM],
                                     start=(k == 0), stop=(k == KT - 1))
                vev = evac.tile([128, 8, DHEAD + 1], BF16, tag="vev", name="vev")
                nc.vector.tensor_copy(
                    vev[:, :, 0:DHEAD],
                    ps[:].rearrange("p (h d) -> p h d", h=8),
                )
                nc.vector.memset(vev[:, :, DHEAD:DHEAD + 1], 1.0)
                for j in range(2, NPAIRS):
                    nc.gpsimd.dma_start(
                        out=agv_in[j][:, :, st, :],
                        in_=vev[:, 2 * (j - 2):2 * (j - 2) + 2, :],
                    )

            # interleave per-pair K and V gathers so pair j's data lands as
            # a unit on the CC engine
            def _agv(j):
                nc.gpsimd.collective_compute(
                    "AllGather", mybir.AluOpType.bypass, replica_groups=groups,
                    ins=[agv_in[j].opt()], outs=[agv_out[j].opt()],
                )

            for j in range(2, NPAIRS):
                _kproj(j)
                _agv(j)
            for mt in range(2, NPAIRS):
                _qproj(mt)

        # prefetch output-projection weights (tiny, avoids tail stall)
        wopool = ctx.enter_context(tc.tile_pool(name="wopool", bufs=1))
        wo_sb = [wopool.tile([128, DIM], BF16, tag=f"wo{k}", name=f"wo{k}") for k in range(NPAIRS)]
        for k in range(NPAIRS):
            nc.sync.dma_start(out=wo_sb[k][:], in_=wo[k])
        bias_sb = wopool.tile([128, DIM], F32, tag="bias", name="bias")
        bo_b = bass.AP(tensor=bo.tensor, offset=bo.offset, ap=[[0, 128]] + bo.ap)
        nc.sync.dma_start(out=bias_sb[:], in_=bo_b)

        # ---------------- Phase B: attention (v1 structure) ----------------
        with ExitStack() as p2:
            epool = p2.enter_context(tc.tile_pool(name="epool", bufs=6))
            npool = p2.enter_context(tc.tile_pool(name="npool", bufs=2))
            psS = p2.enter_context(tc.tile_pool(name="psS", bufs=3, space="PSUM"))
            psO = p2.enter_context(tc.tile_pool(name="psO", bufs=1, space="PSUM"))

            kv_pre = {}

            def _load_pair(j):
                kt1 = ktiles[j % 2]
                kt2 = ktiles2[j % 2]
                nc.sync.dma_start(
                    out=kt1[0:DHEAD, :].rearrange("p (c s) -> p c s", c=N_CORES),
                    in_=agk_out[j][:, 0:DHEAD, :].rearrange("c p s -> p c s"),
                )
                nc.sync.dma_start(
                    out=kt2[0:DHEAD, :].rearrange("p (c s) -> p c s", c=N_CORES),
                    in_=agk_out[j][:, DHEAD:128, :].rearrange("c p s -> p c s"),
                )
                v1 = vpool.tile([128, SEQ // 128, DHEAD + 1], BF16, tag="v1", name="v1")
                v2 = vpool.tile([128, SEQ // 128, DHEAD + 1], BF16, tag="v2", name="v2")
                nc.sync.dma_start(
                    out=v1[:].rearrange("p (c s) d -> p c s d", c=N_CORES),
                    in_=agv_out[j][:, :, 0, :, :].rearrange("c p s d -> p c s d"),
                )
                nc.sync.dma_start(
                    out=v2[:].rearrange("p (c s) d -> p c s d", c=N_CORES),
                    in_=agv_out[j][:, :, 1, :, :].rearrange("c p s d -> p c s d"),
                )
                kv_pre[j] = (kt1, kt2, v1, v2)

            kv_pre[0] = (ktiles[0], ktiles2[0], v1_0, v2_0)
            kv_pre[1] = (ktiles[1], ktiles2[1], v1_1, v2_1)

            for j in range(NPAIRS):
                if 2 <= j + 1 < NPAIRS:
                    _load_pair(j + 1)
                h1, h2 = 2 * j, 2 * j + 1
                kt1, kt2, v1, v2 = kv_pre.pop(j)

                accO1 = npool.tile([DHEAD + 1, NQ], F32, tag="accO1", name="accO1")
                accO2 = npool.tile([DHEAD + 1, NQ], F32, tag="accO2", name="accO2")
                pO1 = pO2 = None
                for g in range(16):  # groups of 2 key-tiles of 128 = 256 keys
                    if g % 8 == 0:
                        pO1 = psO.tile([DHEAD + 1, NQ], F32, tag="po1", name="pO1")
                        pO2 = psO.tile([DHEAD + 1, NQ], F32, tag="po2", name="pO2")
                    pS1 = psS.tile([128, 2, 512], F32, tag="ps", name="pS1")
                    pS2 = psS.tile([128, 2, 512], F32, tag="ps", name="pS2")
                    for i in range(2):
                        kb = g * 2 + i
                        nc.tensor.matmul(
                            pS1[:, i, :], kt1[:, kb * 128:(kb + 1) * 128],
                            qT_sb[h1][:], start=True, stop=True,
                        )
                        nc.tensor.matmul(
                            pS2[:, i, :], kt2[:, kb * 128:(kb + 1) * 128],
                            qT_sb[h2][:], start=True, stop=True,
                        )
                    e1 = epool.tile([128, 2, 512], BF16, tag="e1", name="e1")
                    e2 = epool.tile([128, 2, 512], BF16, tag="e2", name="e2")
                    nc.scalar.activation(e1[:], pS1[:], mybir.ActivationFunctionType.Exp)
                    nc.scalar.activation(e2[:], pS2[:], mybir.ActivationFunctionType.Exp)
                    for i in range(2):
                        kb = g * 2 + i
                        nc.tensor.matmul(pO1[:], v1[:, kb, :], e1[:, i, :],
                                         start=(kb % 16 == 0), stop=(kb % 16 == 15))
                        nc.tensor.matmul(pO2[:], v2[:, kb, :], e2[:, i, :],
                                         start=(kb % 16 == 0), stop=(kb % 16 == 15))
                    if g % 8 == 7:
                        # evacuate psum half into SBUF accumulators; frees the
                        # psum bank so the next half/pair can start immediately
                        if g == 7:
                            nc.vector.tensor_copy(accO1[:], pO1[:])
                            nc.vector.tensor_copy(accO2[:], pO2[:])
                        else:
                            nc.vector.tensor_add(accO1[:], accO1[:], pO1[:])
                            nc.vector.tensor_add(accO2[:], accO2[:], pO2[:])

                # normalize: recip of denominator rows; GpSimd broadcasts
                rec1 = npool.tile([1, NQ], F32, tag="rec1", name="rec1")
                rec2 = npool.tile([1, NQ], F32, tag="rec2", name="rec2")
                nc.vector.reciprocal(rec1[:], accO1[DHEAD:DHEAD + 1, :])
                nc.vector.reciprocal(rec2[:], accO2[DHEAD:DHEAD + 1, :])
                b1 = npool.tile([DHEAD, NQ], F32, tag="b1", name="b1")
                b2 = npool.tile([DHEAD, NQ], F32, tag="b2", name="b2")
                nc.gpsimd.partition_broadcast(b1[:], rec1[:])
                nc.gpsimd.partition_broadcast(b2[:], rec2[:])
                nc.vector.tensor_mul(proj_sb[j][0:DHEAD, :], accO1[0:DHEAD, :], b1[:])
                nc.vector.tensor_mul(proj_sb[j][DHEAD:128, :], accO2[0:DHEAD, :], b2[:])

        # ---------------- Phase C: output projection ----------------
        with ExitStack() as p3:
            opool = p3.enter_context(tc.tile_pool(name="opool", bufs=2))
            psF = p3.enter_context(tc.tile_pool(name="psF", bufs=2, space="PSUM"))

            for qt in range(NQ // 128):
                ps = psF.tile([128, DIM], F32, tag="psf", name="psf")
                for k in range(NPAIRS):
                    lhs = proj_sb[k][:, qt * 128:(qt + 1) * 128]
                    nc.tensor.matmul(ps[:, 0:512], lhs, wo_sb[k][:, 0:512],
                                     start=(k == 0), stop=(k == NPAIRS - 1))
                    nc.tensor.matmul(ps[:, 512:DIM], lhs, wo_sb[k][:, 512:DIM],
                                     start=(k == 0), stop=(k == NPAIRS - 1))
                of = opool.tile([128, DIM], F32, tag="of", name="of")
                nc.vector.tensor_add(of[:], ps[:], bias_sb[:])
                nc.sync.dma_start(out=out[qt * 128:(qt + 1) * 128, :], in_=of[:])

    nc.compile()
    return nc


def kernel(x, W_qkv, W_out, b_out):
    import ml_dtypes

    if "nc" not in _CACHE:
        _CACHE["nc"] = _build()
    nc = _CACHE["nc"]

    x = np.asarray(x, dtype=np.float32)
    W_qkv = np.asarray(W_qkv, dtype=np.float32)
    W_out = np.asarray(W_out, dtype=np.float32)
    b_out = np.asarray(b_out, dtype=np.float32)

    wq_h = np.ascontiguousarray(W_qkv[:, 0:DIM]).reshape(KT, 128, DIM)
    wk_h = np.ascontiguousarray(W_qkv[:, DIM:2 * DIM]).reshape(KT, 128, DIM)
    wv_h = np.ascontiguousarray(W_qkv[:, 2 * DIM:3 * DIM]).reshape(KT, 128, DIM)
    wo_h = np.ascontiguousarray(W_out.astype(ml_dtypes.bfloat16)).reshape(NPAIRS, 128, DIM)
    xT16_h = np.ascontiguousarray(x[0].T.astype(np.float16)).reshape(KT, 128, SEQ)
    wk01_h = np.ascontiguousarray(W_qkv[:, DIM:DIM + 256].astype(np.float16)).reshape(KT, 128, 256)
    wv01_h = np.ascontiguousarray(W_qkv[:, 2 * DIM:2 * DIM + 256].astype(np.float16)).reshape(KT, 128, 256)

    in_maps = []
    for c in range(N_CORES):
        xcT = np.ascontiguousarray(x[0, c * NSH:(c + 1) * NSH, :].T).reshape(KT, 128, NSH)
        in_maps.append({
            "xcT": xcT, "xT16": xT16_h, "wk01": wk01_h, "wv01": wv01_h,
            "wq": wq_h, "wk": wk_h, "wv": wv_h,
            "wo": wo_h, "bo": b_out,
        })

    res = run_bass_kernel_spmd(
        nc, in_maps, list(range(N_CORES)),
        trace=bool(os.environ.get("KERNEL_TRACE")),
    )
    _CACHE["last_exec_time_ns"] = res.exec_time_ns
    out = np.concatenate([res.results[c]["out"] for c in range(N_CORES)], axis=0)
    return out.reshape(1, SEQ, DIM)



# revision 5
# speedup vs baseline: 1.1847x; 1.1847x over previous
"""Trainium2 Bass kernel v5 for nn_Attention (B=1, N=4096, DIM=768, HEADS=12).

Design vs v4 (436us):
- Single shared PSUM plan for the whole kernel: one ring pool of
  2x[128,3,512] f32 slots (6 banks) + pO1/pO2 (2 banks) = 16KB. No pool
  transitions -> no cross-phase drain barriers (v4's first exp was at
  137us because phase-B PSUM reused phase-A addresses).
- Row-packed score matmuls: kt/qT pack a head pair on 128 partitions
  (h1 dims 0-63, h2 dims 64-127); two concurrent K=64 matmuls via
  tile_position (auto-derived from base partitions) -> halves score
  PE time and kt SBUF.
- exp batched to [128,3,512] slots (F=1536) -> 1/3 fewer ACT instrs;
  ACT is the end-to-end bottleneck (~200us busy).
- Pairs 0-2 K/V computed redundantly full-seq (fills PE while CC engine
  does its ~70us init + gathers); pairs 3-5 sharded + per-pair
  AllGathers ordered K3,V3,K4,V4,K5,V5.
- Softmax denominators via ones-column of V (M=65 AV matmuls);
  normalization: DVE copy den row 64->0, reciprocal_approx_fast,
  gpsimd partition_broadcast, DVE mul straight out of PSUM.
- All weights/x in fp16 (bf16 for V/W_out), one DMA'd x^T copy + a
  per-core own-chunk copy.
"""

import os
import sys
from contextlib import ExitStack

import numpy as np

sys.path.insert(0, "/opt/trn_rl_repo")

import concourse.bass as bass  # noqa: E402
import concourse.tile as tile  # noqa: E402
from concourse import bacc, mybir  # noqa: E402
from concourse.bass_utils import run_bass_kernel_spmd  # noqa: E402

N_CORES = 8
DIM = 768
HEADS = 12
SEQ = 4096
DHEAD = 64
NQ = SEQ // N_CORES  # 512 queries per core
NPAIRS = HEADS // 2  # 6 head pairs
KT = DIM // 128  # 6 contraction tiles
NKB = SEQ // 128  # 32 key blocks
F32 = mybir.dt.float32
F16 = mybir.dt.float16
BF16 = mybir.dt.bfloat16
Exp = mybir.ActivationFunctionType.Exp

_CACHE = {}


def _build():
    nc = bacc.Bacc("TRN2", target_bir_lowering=False, debug=False, num_devices=N_CORES)

    xc16 = nc.dram_tensor("xc16", [KT, 128, NQ], F16, kind="ExternalInput").ap()
    xT16 = nc.dram_tensor("xT16", [KT, 128, SEQ], F16, kind="ExternalInput").ap()
    wq = nc.dram_tensor("wq", [KT, 128, DIM], F16, kind="ExternalInput").ap()
    wk = nc.dram_tensor("wk", [KT, 128, DIM], F16, kind="ExternalInput").ap()
    wv = nc.dram_tensor("wv", [KT, 128, DIM], F16, kind="ExternalInput").ap()
    wo = nc.dram_tensor("wo", [NPAIRS, 128, DIM], BF16, kind="ExternalInput").ap()
    bo = nc.dram_tensor("bo", [DIM], F32, kind="ExternalInput").ap()
    out = nc.dram_tensor("out", [NQ, DIM], F32, kind="ExternalOutput").ap()

    # collective bounce buffers for pairs 3-5
    agk_in = {p: nc.dram_tensor(f"agk_in{p}", [128, NQ], F16).ap() for p in range(3, 6)}
    agk_out = {
        p: nc.dram_tensor(f"agk_out{p}", [N_CORES, 128, NQ], F16, addr_space="Shared").ap()
        for p in range(3, 6)
    }
    agv_in = {
        p: nc.dram_tensor(f"agv_in{p}", [128, 4, 2, DHEAD + 1], BF16).ap()
        for p in range(3, 6)
    }
    agv_out = {
        p: nc.dram_tensor(
            f"agv_out{p}", [N_CORES, 128, 4, 2, DHEAD + 1], BF16, addr_space="Shared"
        ).ap()
        for p in range(3, 6)
    }
    groups = [list(range(N_CORES))]

    with ExitStack() as ctx:
        tc = ctx.enter_context(tile.TileContext(nc))

        persist = ctx.enter_context(tc.tile_pool(name="persist", bufs=1))
        ring = ctx.enter_context(tc.tile_pool(name="ring", bufs=2, space="PSUM"))
        psO = ctx.enter_context(tc.tile_pool(name="psO", bufs=1, space="PSUM"))
        evac = ctx.enter_context(tc.tile_pool(name="evac", bufs=3))
        epool = ctx.enter_context(tc.tile_pool(name="epool", bufs=5))
        npool = ctx.enter_context(tc.tile_pool(name="npool", bufs=2))

        # ---- persistent SBUF ----
        qT_sb = [persist.tile([128, NQ], F16, tag=f"qt{p}", name=f"qt{p}") for p in range(NPAIRS)]
        proj_sb = [persist.tile([128, NQ], BF16, tag=f"proj{p}", name=f"proj{p}") for p in range(NPAIRS)]
        ktiles = [persist.tile([128, SEQ], F16, tag=f"ktile{i}", name=f"ktile{i}") for i in range(2)]
        vtiles = [
            persist.tile([128, NKB, 2, DHEAD + 1], BF16, tag=f"vtile{i}", name=f"vtile{i}")
            for i in range(3)
        ]
        xc_sb = [persist.tile([128, NQ], F16, tag=f"xc{k}", name=f"xc{k}") for k in range(KT)]
        x16_sb = [persist.tile([128, SEQ], F16, tag=f"x16_{k}", name=f"x16_{k}") for k in range(KT)]
        wq_sb = [persist.tile([128, DIM], F16, tag=f"wq{k}", name=f"wq{k}") for k in range(KT)]
        wk_sb = [persist.tile([128, DIM], F16, tag=f"wk{k}", name=f"wk{k}") for k in range(KT)]
        wv_sb = [persist.tile([128, DIM], F16, tag=f"wv{k}", name=f"wv{k}") for k in range(KT)]
        wo_sb = [persist.tile([128, DIM], BF16, tag=f"wo{p}", name=f"wo{p}") for p in range(NPAIRS)]
        bias_sb = persist.tile([128, DIM], F32, tag="bias", name="bias")

        # ---- DMAs in need-order ----
        for k in range(KT):
            nc.sync.dma_start(out=xc_sb[k][:], in_=xc16[k])
        for k in range(KT):
            nc.sync.dma_start(out=wk_sb[k][:], in_=wk[k])
            nc.sync.dma_start(out=wv_sb[k][:], in_=wv[k])
            nc.sync.dma_start(out=wq_sb[k][:], in_=wq[k])
        for k in range(KT):
            nc.sync.dma_start(out=x16_sb[k][:], in_=xT16[k])
        for p in range(NPAIRS):
            nc.sync.dma_start(out=wo_sb[p][:], in_=wo[p])
        bo_b = bass.AP(tensor=bo.tensor, offset=bo.offset, ap=[[0, 128]] + bo.ap)
        nc.sync.dma_start(out=bias_sb[:], in_=bo_b)

        # ones columns for locally-computed V (pairs 0-2); gathered pairs
        # (3-5) bring their ones through the collective.
        for i in range(3):
            nc.vector.memset(vtiles[i][:, :, :, DHEAD:DHEAD + 1], 1.0)

        # ---- sharded K/V projections for pairs 3-5 + AllGathers ----
        def _kproj(p):
            ps = ring.tile([128, 3, NQ], F32, tag="ps", name=f"psk{p}")
            for k in range(KT):
                nc.tensor.matmul(
                    ps[:, 0, :], wk_sb[k][:, p * 128:(p + 1) * 128], xc_sb[k][:],
                    start=(k == 0), stop=(k == KT - 1),
                )
            kev = evac.tile([128, NQ], F16, tag="kev", name="kev")
            nc.vector.tensor_copy(kev[:], ps[:, 0, :])
            nc.gpsimd.dma_start(out=agk_in[p], in_=kev[:])
            nc.gpsimd.collective_compute(
                "AllGather", mybir.AluOpType.bypass, replica_groups=groups,
                ins=[agk_in[p].opt()], outs=[agk_out[p].opt()],
            )

        def _agv(p):
            nc.gpsimd.collective_compute(
                "AllGather", mybir.AluOpType.bypass, replica_groups=groups,
                ins=[agv_in[p].opt()], outs=[agv_out[p].opt()],
            )

        _kproj(3)
        # V own-chunk for pairs 3-5, staged per-pair
        for st in range(4):
            ps = ring.tile([128, 3, NQ], F32, tag="ps", name=f"psv{st}")
            for k in range(KT):
                nc.tensor.matmul(
                    ps[:, 0, 0:384], xc_sb[k][:, st * 128:(st + 1) * 128],
                    wv_sb[k][:, 384:DIM], start=(k == 0), stop=(k == KT - 1),
                )
            vev = evac.tile([128, 6, DHEAD + 1], BF16, tag="vev", name="vev")
            nc.vector.tensor_copy(
                vev[:, :, 0:DHEAD],
                ps[:, 0, 0:384].rearrange("p (h d) -> p h d", h=6),
            )
            nc.vector.memset(vev[:, :, DHEAD:DHEAD + 1], 1.0)
            for p in range(3, 6):
                nc.gpsimd.dma_start(
                    out=agv_in[p][:, st, :, :],
                    in_=vev[:, 2 * (p - 3):2 * (p - 3) + 2, :],
                )
        _agv(3)
        _kproj(4)
        _agv(4)
        _kproj(5)
        _agv(5)

        # ---- Q projections (packed per pair) ----
        for p in range(NPAIRS):
            ps = ring.tile([128, 3, NQ], F32, tag="ps", name=f"psq{p}")
            for k in range(KT):
                nc.tensor.matmul(
                    ps[:, 0, :], wq_sb[k][:, p * 128:(p + 1) * 128], xc_sb[k][:],
                    start=(k == 0), stop=(k == KT - 1),
                )
            nc.vector.tensor_copy(qT_sb[p][:], ps[:, 0, :])

        # ---- full-seq K for a local pair (j in 0..2) into kt ----
        def _kfull_chunks(j, kt_dst):
            chunks = []
            for ch in range(8):
                def _do(ch=ch):
                    ps = ring.tile([128, 3, NQ], F32, tag="ps", name=f"pskf{j}_{ch}")
                    for k in range(KT):
                        nc.tensor.matmul(
                            ps[:, 0, :], wk_sb[k][:, j * 128:(j + 1) * 128],
                            x16_sb[k][:, ch * 512:(ch + 1) * 512],
                            start=(k == 0), stop=(k == KT - 1),
                        )
                    nc.vector.tensor_copy(kt_dst[:, ch * 512:(ch + 1) * 512], ps[:, 0, :])
                chunks.append(_do)
            return chunks

        # ---- full-seq V for pairs 0-2 (N=384 streams), 2 seq-tiles/chunk ----
        def _v012_chunks():
            chunks = []
            for st0 in range(0, NKB, 2):
                def _do(st0=st0):
                    ps = ring.tile([128, 3, NQ], F32, tag="ps", name=f"psv012_{st0}")
                    for s2 in range(2):
                        for k in range(KT):
                            nc.tensor.matmul(
                                ps[:, s2, 0:384],
                                x16_sb[k][:, (st0 + s2) * 128:(st0 + s2 + 1) * 128],
                                wv_sb[k][:, 0:384],
                                start=(k == 0), stop=(k == KT - 1),
                            )
                    for p in range(3):
                        nc.vector.tensor_copy(
                            vtiles[p][:, st0:st0 + 2, :, 0:DHEAD],
                            ps[:, 0:2, p * 128:(p + 1) * 128].rearrange(
                                "p s (h d) -> p s h d", h=2),
                        )
                chunks.append(_do)
            return chunks

        # kfull(0) fully before attention
        for c in _kfull_chunks(0, ktiles[0]):
            c()

        # Deferred projection work, emitted inside pair-0/1 attention windows.
        # Producers MUST be emitted before their consumers (the dep tracker
        # links reads to already-emitted writers only):
        #  - pair 0 slot s consumes V012 chunk (3s+2)//4 at most; emit 2
        #    upfront + 3 per 4 slots to stay >=1 chunk ahead.
        #  - kfull1 fully emitted during pair 0; kfull2 during pair 1.
        v012_work = _v012_chunks()  # 16 chunks
        k1_work = _kfull_chunks(1, ktiles[1])  # 8
        k2_work = _kfull_chunks(2, ktiles[0])  # 8
        for _ in range(2):
            v012_work.pop(0)()

        def _pair0_pre_slot(s):
            target = min(14, (3 * (s + 1)) // 4)  # chunks beyond the 2 upfront
            while 14 - len(v012_work) < target:
                v012_work.pop(0)()
            k1_target = min(8, max(0, (s - 2) // 2))
            while 8 - len(k1_work) < k1_target:
                k1_work.pop(0)()

        def _pair1_pre_slot(s):
            while v012_work:
                v012_work.pop(0)()
            while k1_work:
                k1_work.pop(0)()
            k2_target = min(8, (s + 1) // 2)
            while 8 - len(k2_work) < k2_target:
                k2_work.pop(0)()

        pre_slot_hooks = {0: _pair0_pre_slot, 1: _pair1_pre_slot}

        # ---- gathered pair loads (pairs 3-5) ----
        def _load_pair(j):
            p = j  # pair index 3..5
            kt_dst = ktiles[j % 2]
            nc.sync.dma_start(
                out=kt_dst[:].rearrange("p (c s) -> p c s", c=N_CORES),
                in_=agk_out[p].rearrange("c p s -> p c s"),
            )
            v_dst = vtiles[j % 3]
            nc.sync.dma_start(
                out=v_dst[:].rearrange("p (c s) h d -> p c s h d", c=N_CORES),
                in_=agv_out[p].rearrange("c p s h d -> p c s h d"),
            )

        # ---- attention per pair ----
        units = [(kb, h) for kb in range(NKB) for h in range(2)]  # 64 units
        nslots = (len(units) + 2) // 3  # 22

        def _attention(j, kt_cur, v_cur):
            pO1 = psO.tile([DHEAD + 1, NQ], F32, tag="po1", name=f"pO1_{j}")
            pO2 = psO.tile([DHEAD + 1, NQ], F32, tag="po2", name=f"pO2_{j}")
            pOs = (pO1, pO2)
            slots_e = [None] * nslots
            work_i = 0

            def emit_scores(s):
                su = units[3 * s:3 * s + 3]
                ps = ring.tile([128, 3, NQ], F32, tag="ps", name=f"sc{j}_{s}")
                for i, (kb, h) in enumerate(su):
                    nc.tensor.matmul(
                        ps[:, i, :],
                        kt_cur[64 * h:64 * (h + 1), kb * 128:(kb + 1) * 128],
                        qT_sb[j][64 * h:64 * (h + 1), :],
                        start=True, stop=True,
                    )
                e = epool.tile([128, 3, NQ], BF16, tag="e", name=f"e{j}_{s}")
                nc.scalar.activation(e[:, 0:len(su), :], ps[:, 0:len(su), :], Exp)
                slots_e[s] = e

            def emit_av(s):
                su = units[3 * s:3 * s + 3]
                e = slots_e[s]
                for i, (kb, h) in enumerate(su):
                    nc.tensor.matmul(
                        pOs[h][:], v_cur[:, kb, h, :], e[:, i, :],
                        start=(kb == 0), stop=(kb == NKB - 1),
                    )

            hook = pre_slot_hooks.get(j)
            for s in range(nslots):
                if hook is not None:
                    hook(s)
                emit_scores(s)
                if s >= 1:
                    emit_av(s - 1)
            emit_av(nslots - 1)
            # safety flush before pair 2 needs kt0/vtiles
            if j == 1:
                while v012_work:
                    v012_work.pop(0)()
                while k1_work:
                    k1_work.pop(0)()
                while k2_work:
                    k2_work.pop(0)()

            # normalization: den row 64 -> partition 0, approx recip,
            # broadcast, multiply straight out of PSUM.
            den1 = npool.tile([1, NQ], F32, tag="den1", name=f"den1_{j}")
            den2 = npool.tile([1, NQ], F32, tag="den2", name=f"den2_{j}")
            nc.vector.tensor_copy(den1[:], pO1[DHEAD:DHEAD + 1, :])
            nc.vector.tensor_copy(den2[:], pO2[DHEAD:DHEAD + 1, :])
            rec1 = npool.tile([1, NQ], F32, tag="rec1", name=f"rec1_{j}")
            rec2 = npool.tile([1, NQ], F32, tag="rec2", name=f"rec2_{j}")
            nc.vector.reciprocal_approx_fast(rec1[:], den1[:])
            nc.vector.reciprocal_approx_fast(rec2[:], den2[:])
            b1 = npool.tile([DHEAD, NQ], F32, tag="b1", name=f"b1_{j}")
            b2 = npool.tile([DHEAD, NQ], F32, tag="b2", name=f"b2_{j}")
            nc.gpsimd.partition_broadcast(b1[:], rec1[:])
            nc.gpsimd.partition_broadcast(b2[:], rec2[:])
            nc.vector.tensor_mul(proj_sb[j][0:DHEAD, :], pO1[0:DHEAD, :], b1[:])
            nc.vector.tensor_mul(proj_sb[j][DHEAD:128, :], pO2[0:DHEAD, :], b2[:])

        for j in range(NPAIRS):
            if 3 <= j + 1 <= 5:
                _load_pair(j + 1)
            kt_cur = ktiles[j % 2]
            v_cur = vtiles[j % 3]
            _attention(j, kt_cur, v_cur)

        # ---- output projection ----
        opool = ctx.enter_context(tc.tile_pool(name="opool", bufs=2))
        for qt in range(NQ // 128):
            ps = ring.tile([128, 3, NQ], F32, tag="ps", name=f"psf{qt}")
            for p in range(NPAIRS):
                lhs = proj_sb[p][:, qt * 128:(qt + 1) * 128]
                nc.tensor.matmul(ps[:, 0, :], lhs, wo_sb[p][:, 0:512],
                                 start=(p == 0), stop=(p == NPAIRS - 1))
                nc.tensor.matmul(ps[:, 1, 0:256], lhs, wo_sb[p][:, 512:DIM],
                                 start=(p == 0), stop=(p == NPAIRS - 1))
            of = opool.tile([128, DIM], F32, tag="of", name="of")
            nc.vector.tensor_add(of[:, 0:512], ps[:, 0, :], bias_sb[:, 0:512])
            nc.vector.tensor_add(of[:, 512:DIM], ps[:, 1, 0:256], bias_sb[:, 512:DIM])
            nc.sync.dma_start(out=out[qt * 128:(qt + 1) * 128, :], in_=of[:])

    nc.compile()
    return nc


def kernel(x, W_qkv, W_out, b_out):
    import ml_dtypes

    if "nc" not in _CACHE:
        _CACHE["nc"] = _build()
    nc = _CACHE["nc"]

    x = np.asarray(x, dtype=np.float32)
    W_qkv = np.asarray(W_qkv, dtype=np.float32)
    W_out = np.asarray(W_out, dtype=np.float32)
    b_out = np.asarray(b_out, dtype=np.float32)

    wq_h = np.ascontiguousarray(W_qkv[:, 0:DIM].astype(np.float16)).reshape(KT, 128, DIM)
    wk_h = np.ascontiguousarray(W_qkv[:, DIM:2 * DIM].astype(np.float16)).reshape(KT, 128, DIM)
    wv_h = np.ascontiguousarray(W_qkv[:, 2 * DIM:3 * DIM].astype(np.float16)).reshape(KT, 128, DIM)
    wo_h = np.ascontiguousarray(W_out.astype(ml_dtypes.bfloat16)).reshape(NPAIRS, 128, DIM)
    xT16_h = np.ascontiguousarray(x[0].T.astype(np.float16)).reshape(KT, 128, SEQ)

    in_maps = []
    for c in range(N_CORES):
        xc16_h = np.ascontiguousarray(
            x[0, c * NQ:(c + 1) * NQ, :].T.astype(np.float16)
        ).reshape(KT, 128, NQ)
        in_maps.append({
            "xc16": xc16_h, "xT16": xT16_h,
            "wq": wq_h, "wk": wk_h, "wv": wv_h,
            "wo": wo_h, "bo": b_out,
        })

    res = run_bass_kernel_spmd(
        nc, in_maps, list(range(N_CORES)),
        trace=bool(os.environ.get("KERNEL_TRACE")),
    )
    _CACHE["last_exec_time_ns"] = res.exec_time_ns
    out = np.concatenate([res.results[c]["out"] for c in range(N_CORES)], axis=0)
    return out.reshape(1, SEQ, DIM)
